# revision 9
# baseline (speedup 1.0000x reference)
"""Trainium2 Bass kernel for AscendRejectionSampler (speculative-decoding
rejection sampling), SPMD across 8 NeuronCores.

Strategy (data parallel over requests/tokens, no collectives):

The reference output needs, per request, the accepted prefix of draft tokens
plus ONE "repair" token at the first rejected position:
  - greedy requests emit argmax(target_probs[row]) there,
  - non-greedy requests emit argmax(relu(t-d)/q) ("recovered") there.
Accept bits need only single-element gathers t[i, dtid], d[i, dtid] (random)
or the row argmax itself (greedy).  So the expensive full-vocab scans are
needed for ~1 row per request, not for every token.

Phase 1 (NEFF 1): per-token 64-wide chunks containing dtid are staged by the
host; the device extracts t_at/d_at via masked tensor_tensor_reduce and
computes the random-accept bits exactly ((d>0) & (t >= u*d), IEEE f32).

Host: per-request scan over accept bits (control flow) picks the first
rejected position; greedy requests start with frontier position 0.

Phase 2 (NEFF 2): the needed rows are staged split 16-partitions-per-row
(8 rows per 128-partition group, 2000 vocab/partition).  Ratio rows run
(t - d) / q with exact IEEE divide on the vector engine, then per-partition
top-8 (max) + indices (max_index).  Greedy rows take top-8 of raw t.  The
host merges the 128 candidates per row and resolves the exact first-index
argmax (ties included) by re-checking only the few candidate elements.
If a greedy frontier row is *accepted* (P ~ 1/32000 per row), the frontier
advances and NEFF 2 reruns for those few rows.

All O(N*V) data processing happens on-device; the host does control flow,
staging/unstaging, and O(candidates) exact tie resolution.
"""

import sys

if '/opt/trn_rl_repo' not in sys.path:
    sys.path.insert(0, '/opt/trn_rl_repo')

import numpy as np

NCORES = 8
PLACEHOLDER = -1
VP = 2000          # vocab elements per partition in phase 2 (16 partitions/row)
PARTS_PER_ROW = 16
ROWS_PER_GROUP = 128 // PARTS_PER_ROW  # 8

# Collected exec_time_ns per NEFF execution when profiling is enabled.
PROFILE = False
LAST_EXEC_NS = []

_BUILT = {}


def _bass_mods():
    import concourse.mybir as mybir
    from concourse import bass
    from concourse.bass_utils import run_bass_kernel_spmd
    return mybir, bass, run_bass_kernel_spmd


def _maybe_install_ntff_hook():
    """Register the axon NTFF profile hook if the image lacks antenv.axon_hooks."""
    import types
    try:
        import antenv.axon_hooks  # noqa: F401
        return
    except ImportError:
        pass
    import antenv
    mod = types.ModuleType('antenv.axon_hooks')
    _h = [None]
    mod.set_axon_ntff_profile_hook = lambda h: _h.__setitem__(0, h)
    mod.get_axon_ntff_profile_hook = lambda: _h[0]
    sys.modules['antenv.axon_hooks'] = mod
    antenv.axon_hooks = mod
    try:
        from trn_agent_boot.trn_boot import _ntff_profile_via_ctypes
        mod.set_axon_ntff_profile_hook(
            _ntff_profile_via_ctypes('/opt/axon/libaxon_pjrt.so'))
    except Exception:
        pass


def _run(nc, in_maps):
    _, _, run_bass_kernel_spmd = _bass_mods()
    if PROFILE:
        _maybe_install_ntff_hook()
        res = run_bass_kernel_spmd(nc, in_maps, core_ids=list(range(NCORES)),
                                   trace=True)
        if res.exec_time_ns is not None:
            LAST_EXEC_NS.append(res.exec_time_ns)
        return res.results
    res = run_bass_kernel_spmd(nc, in_maps, core_ids=list(range(NCORES)))
    return res.results


# --------------------------------------------------------------------------
# Phase 1: single-element gathers + random-accept bits
# --------------------------------------------------------------------------

def _build_phase1(SL1):
    """Per core: [128, SL1] token slots. Inputs ct/cd: [128, SL1*64] staged
    64-elem chunks of t/d containing dtid; em: one-hot mask; uu: uniforms.
    Outputs ta/da (gathered values) and bit (random accept)."""
    key = ('p1', SL1)
    if key in _BUILT:
        return _BUILT[key]
    mybir, bass, _ = _bass_mods()
    DT = mybir.dt.float32
    F = SL1 * 64
    nc = bass.Bass()
    ct = nc.declare_dram_parameter("ct", [128, F], DT, isOutput=False)
    cd = nc.declare_dram_parameter("cd", [128, F], DT, isOutput=False)
    em = nc.declare_dram_parameter("em", [128, F], DT, isOutput=False)
    uu = nc.declare_dram_parameter("uu", [128, SL1], DT, isOutput=False)
    ta_o = nc.declare_dram_parameter("ta", [128, SL1], DT, isOutput=True)
    da_o = nc.declare_dram_parameter("da", [128, SL1], DT, isOutput=True)
    bit_o = nc.declare_dram_parameter("bit", [128, SL1], DT, isOutput=True)

    with (
        nc.Block() as block,
        nc.semaphore("dma_sem") as dma_sem,
        nc.semaphore("v_sem") as v_sem,
        nc.sbuf_tensor("ct_sb", [128, F], DT) as ct_sb,
        nc.sbuf_tensor("cd_sb", [128, F], DT) as cd_sb,
        nc.sbuf_tensor("em_sb", [128, F], DT) as em_sb,
        nc.sbuf_tensor("uu_sb", [128, SL1], DT) as uu_sb,
        nc.sbuf_tensor("junk_sb", [128, 64], DT) as junk_sb,
        nc.sbuf_tensor("ta_sb", [128, SL1], DT) as ta_sb,
        nc.sbuf_tensor("da_sb", [128, SL1], DT) as da_sb,
        nc.sbuf_tensor("t1_sb", [128, SL1], DT) as t1_sb,
        nc.sbuf_tensor("t2_sb", [128, SL1], DT) as t2_sb,
        nc.sbuf_tensor("bit_sb", [128, SL1], DT) as bit_sb,
    ):
        @block.sync
        def _(sync):
            sync.dma_start(out=ct_sb[:, :], in_=ct[:, :]).then_inc(dma_sem, 16)
            sync.dma_start(out=cd_sb[:, :], in_=cd[:, :]).then_inc(dma_sem, 16)
            sync.dma_start(out=em_sb[:, :], in_=em[:, :]).then_inc(dma_sem, 16)
            sync.dma_start(out=uu_sb[:, :], in_=uu[:, :]).then_inc(dma_sem, 16)
            sync.wait_ge(v_sem, 1)
            sync.dma_start(out=ta_o[:, :], in_=ta_sb[:, :]).then_inc(dma_sem, 16)
            sync.dma_start(out=da_o[:, :], in_=da_sb[:, :]).then_inc(dma_sem, 16)
            sync.dma_start(out=bit_o[:, :], in_=bit_sb[:, :]).then_inc(dma_sem, 16)
            sync.wait_ge(dma_sem, 112)

        @block.vector
        def _(v):
            v.wait_ge(dma_sem, 64)
            A = mybir.AluOpType
            for sl in range(SL1):
                c = slice(sl * 64, (sl + 1) * 64)
                v.scalar_tensor_tensor(
                    junk_sb[:, :], ct_sb[:, c], 1.0, em_sb[:, c],
                    A.mult, A.mult, accum_out=ta_sb[:, sl:sl + 1])
                v.drain()
                v.scalar_tensor_tensor(
                    junk_sb[:, :], cd_sb[:, c], 1.0, em_sb[:, c],
                    A.mult, A.mult, accum_out=da_sb[:, sl:sl + 1])
                v.drain()
            # t1 = (da > 0)
            v.tensor_scalar(t1_sb[:, :], da_sb[:, :], 0.0, None, A.is_gt)
            v.drain()
            # t2 = u * da
            v.tensor_tensor(t2_sb[:, :], uu_sb[:, :], da_sb[:, :], A.mult)
            v.drain()
            # t2 = (ta >= t2)
            v.tensor_tensor(t2_sb[:, :], ta_sb[:, :], t2_sb[:, :], A.is_ge)
            v.drain()
            v.tensor_tensor(bit_sb[:, :], t1_sb[:, :], t2_sb[:, :],
                            A.logical_and)
            v.drain()
            v.sem_inc(v_sem, 1)

    _BUILT[key] = nc
    return nc


# --------------------------------------------------------------------------
# Phase 2: row argmax candidates (top-8 per 16-partition slice of each row)
# --------------------------------------------------------------------------

def _build_phase2(RG, TG):
    """Per core: RG groups of 8 ratio rows ((t-d)/q) and TG groups of 8
    plain-t rows. Each row is split over 16 partitions x VP elements.
    Outputs per-partition top-8 values + indices for every group."""
    key = ('p2', RG, TG)
    if key in _BUILT:
        return _BUILT[key]
    mybir, bass, _ = _bass_mods()
    DT = mybir.dt.float32
    DU = mybir.dt.uint32
    FR = RG * VP
    FT = TG * VP
    nc = bass.Bass()
    t2r = nc.declare_dram_parameter("t2r", [128, FR], DT, isOutput=False)
    d2r = nc.declare_dram_parameter("d2r", [128, FR], DT, isOutput=False)
    q2r = nc.declare_dram_parameter("q2r", [128, FR], DT, isOutput=False)
    t2t = nc.declare_dram_parameter("t2t", [128, FT], DT, isOutput=False)
    m8r_o = nc.declare_dram_parameter("m8r", [128, RG * 8], DT, isOutput=True)
    i8r_o = nc.declare_dram_parameter("i8r", [128, RG * 8], DU, isOutput=True)
    m8t_o = nc.declare_dram_parameter("m8t", [128, TG * 8], DT, isOutput=True)
    i8t_o = nc.declare_dram_parameter("i8t", [128, TG * 8], DU, isOutput=True)

    with (
        nc.Block() as block,
        nc.semaphore("dma_sem") as dma_sem,
        nc.semaphore("v_sem") as v_sem,
        nc.sbuf_tensor("t2t_sb", [128, FT], DT) as t2t_sb,
        nc.sbuf_tensor("t2r_sb", [128, FR], DT) as t2r_sb,
        nc.sbuf_tensor("d2r_sb", [128, FR], DT) as d2r_sb,
        nc.sbuf_tensor("q2r_sb", [128, FR], DT) as q2r_sb,
        nc.sbuf_tensor("s_sb", [128, FR], DT) as s_sb,
        nc.sbuf_tensor("m8r_sb", [128, RG * 8], DT) as m8r_sb,
        nc.sbuf_tensor("i8r_sb", [128, RG * 8], DU) as i8r_sb,
        nc.sbuf_tensor("m8t_sb", [128, TG * 8], DT) as m8t_sb,
        nc.sbuf_tensor("i8t_sb", [128, TG * 8], DU) as i8t_sb,
    ):
        @block.sync
        def _(sync):
            sync.dma_start(out=t2t_sb[:, :], in_=t2t[:, :]).then_inc(dma_sem, 16)
            sync.dma_start(out=t2r_sb[:, :], in_=t2r[:, :]).then_inc(dma_sem, 16)
            sync.dma_start(out=d2r_sb[:, :], in_=d2r[:, :]).then_inc(dma_sem, 16)
            sync.dma_start(out=q2r_sb[:, :], in_=q2r[:, :]).then_inc(dma_sem, 16)
            sync.wait_ge(v_sem, 1)
            sync.dma_start(out=m8t_o[:, :], in_=m8t_sb[:, :]).then_inc(dma_sem, 16)
            sync.dma_start(out=i8t_o[:, :], in_=i8t_sb[:, :]).then_inc(dma_sem, 16)
            sync.wait_ge(v_sem, 2)
            sync.dma_start(out=m8r_o[:, :], in_=m8r_sb[:, :]).then_inc(dma_sem, 16)
            sync.dma_start(out=i8r_o[:, :], in_=i8r_sb[:, :]).then_inc(dma_sem, 16)
            sync.wait_ge(dma_sem, 128)

        @block.vector
        def _(v):
            A = mybir.AluOpType
            # t pipe first: only needs t2t (first DMA)
            v.wait_ge(dma_sem, 16)
            for g in range(TG):
                v.max(m8t_sb[:, g * 8:(g + 1) * 8],
                      t2t_sb[:, g * VP:(g + 1) * VP])
            v.drain()
            for g in range(TG):
                v.max_index(i8t_sb[:, g * 8:(g + 1) * 8],
                            m8t_sb[:, g * 8:(g + 1) * 8],
                            t2t_sb[:, g * VP:(g + 1) * VP])
            v.drain()
            v.sem_inc(v_sem, 1)
            # ratio pipe: s = (t - d) * (1/q); vector.reciprocal is exact
            # (0 ulp vs IEEE 1/x), so the surrogate is within ~1.5 ulp of
            # the reference's (t-d)/q — the host candidate window covers it.
            v.wait_ge(dma_sem, 48)
            v.tensor_tensor(s_sb[:, :], t2r_sb[:, :], d2r_sb[:, :], A.subtract)
            v.drain()
            v.wait_ge(dma_sem, 64)
            v.reciprocal(t2r_sb[:, :], q2r_sb[:, :])
            v.drain()
            v.tensor_tensor(d2r_sb[:, :], s_sb[:, :], t2r_sb[:, :], A.mult)
            v.drain()
            for g in range(RG):
                v.max(m8r_sb[:, g * 8:(g + 1) * 8],
                      d2r_sb[:, g * VP:(g + 1) * VP])
            v.drain()
            for g in range(RG):
                v.max_index(i8r_sb[:, g * 8:(g + 1) * 8],
                            m8r_sb[:, g * 8:(g + 1) * 8],
                            d2r_sb[:, g * VP:(g + 1) * VP])
            v.drain()
            v.sem_inc(v_sem, 1)

    _BUILT[key] = nc
    return nc


# --------------------------------------------------------------------------
# Host-side exact candidate resolution
# --------------------------------------------------------------------------

def _resolve_row(vals, idxs, exact_fn, rescan_fn, reltol=1e-4):
    """vals/idxs: [16, 8] per-partition top-8 (desc) of one row's surrogate,
    idxs are positions within each partition's VP slice. Returns the exact
    first-index argmax of the true row.

    exact_fn(vocab_idx_array) -> exact f32 values (reference arithmetic).
    rescan_fn() -> exact full-row argmax fallback (rare)."""
    vocab = (np.arange(PARTS_PER_ROW)[:, None] * VP + idxs).reshape(-1)
    flat = vals.reshape(-1)
    ms = float(flat.max())
    thr = ms - abs(ms) * reltol
    # truncation guard: a partition whose 8th-best still clears the threshold
    # may have deeper candidates we did not see
    if np.any(vals[:, 7] >= thr):
        return rescan_fn()
    # duplicate guard: equal surrogate values inside one partition's top-8
    # near the max — max_index tie semantics may hide one of the positions
    if np.any((vals[:, :-1] == vals[:, 1:]) & (vals[:, 1:] >= thr)):
        return rescan_fn()
    sel = flat >= thr
    cand_v = vocab[sel]
    exact = exact_fn(cand_v)
    me = exact.max()
    if not (me > 0.0):
        # all-candidates <= 0: the true argmax may be an arbitrary early
        # zero/subnormal position the surrogate can't distinguish
        return rescan_fn()
    winners = cand_v[exact == me]
    if len(winners) > 1:
        # exact ties across candidates: be conservative about capture
        return rescan_fn()
    return int(winners[0])


# --------------------------------------------------------------------------
# The kernel
# --------------------------------------------------------------------------

def kernel(**inputs):
    t = np.ascontiguousarray(np.asarray(inputs['target_probs'], dtype=np.float32))
    d = np.ascontiguousarray(np.asarray(inputs['draft_probs'], dtype=np.float32))
    q = np.ascontiguousarray(np.asarray(inputs['q'], dtype=np.float32))
    u = np.asarray(inputs['uniform_probs'], dtype=np.float32)
    cu = np.asarray(inputs['cu_num_draft_tokens']).astype(np.int64)
    dtid = np.asarray(inputs['draft_token_ids']).astype(np.int64)
    bonus = np.asarray(inputs['bonus_token_ids']).astype(np.int32)
    greedy = np.asarray(inputs['is_greedy']).astype(bool)
    S = int(np.asarray(inputs['max_spec_len']))

    N, V = t.shape
    B = cu.shape[0]
    assert V % (PARTS_PER_ROW * VP) == 0 or V == PARTS_PER_ROW * VP, \
        f"V={V} not supported"
    starts = np.concatenate([[0], cu[:-1]]).astype(np.int64)
    lens = (cu - starts).astype(np.int64)
    tok_req = np.searchsorted(cu, np.arange(N), side='right')

    # ---------------- phase 1 ----------------
    T1 = -(-N // NCORES)                   # tokens per core
    SL1 = max(2, -(-T1 // 128))            # 128-token slots per core
    nc1 = _build_phase1(SL1)

    F = SL1 * 64
    ct_h = np.zeros((NCORES, 128, F), np.float32)
    cd_h = np.zeros((NCORES, 128, F), np.float32)
    em_h = np.zeros((NCORES, 128, F), np.float32)
    uu_h = np.zeros((NCORES, 128, SL1), np.float32)

    ii = np.arange(N)
    core_of = ii // T1
    loc = ii - core_of * T1
    p_of = loc % 128
    sl_of = loc // 128
    base = (dtid // 64) * 64
    off = (dtid % 64).astype(np.int64)
    # chunk gathers (contiguous 64-element slices -> cheap row slicing)
    cols = base[:, None] + np.arange(64)[None, :]
    tch = np.take_along_axis(t, cols, axis=1)
    dch = np.take_along_axis(d, cols, axis=1)
    # vectorized scatter into staging arrays
    col0 = sl_of * 64
    for k in range(64):
        ct_h[core_of, p_of, col0 + k] = tch[:, k]
        cd_h[core_of, p_of, col0 + k] = dch[:, k]
    em_h[core_of, p_of, col0 + off] = 1.0
    uu_h[core_of, p_of, sl_of] = u

    in_maps = [dict(ct=ct_h[c], cd=cd_h[c], em=em_h[c], uu=uu_h[c])
               for c in range(NCORES)]
    res1 = _run(nc1, in_maps)

    ta = np.zeros(N, np.float32)
    da = np.zeros(N, np.float32)
    bits = np.zeros(N, bool)
    for c in range(NCORES):
        lo, hi = c * T1, min(N, (c + 1) * T1)
        n = hi - lo
        ta[lo:hi] = res1[c]['ta'].T.reshape(-1)[:n]
        da[lo:hi] = res1[c]['da'].T.reshape(-1)[:n]
        bits[lo:hi] = res1[c]['bit'].T.reshape(-1)[:n] != 0.0

    # ---------------- host scan (control flow) ----------------
    # random requests: first rejected position from accept bits
    first_rej = np.full(B, -1, np.int64)   # -1 = no rejection so far
    resolved_tok = np.full(B, PLACEHOLDER, np.int64)
    need_ratio = []                        # (req, token_row)
    frontier = {}                          # greedy req -> current position
    for r in range(B):
        s0, L = starts[r], lens[r]
        if greedy[r]:
            frontier[r] = 0
        else:
            rej = np.nonzero(~bits[s0:s0 + L])[0]
            if len(rej):
                first_rej[r] = rej[0]
                need_ratio.append((r, int(s0 + rej[0])))

    # exact reference arithmetic helpers (single-element touches only)
    def ratio_exact(i, r):
        def f(vs):
            num = np.maximum(t[i, vs] - d[i, vs], np.float32(0.0))
            return (num / q[r, vs]).astype(np.float32)
        return f

    def t_exact(i):
        def f(vs):
            return t[i, vs]
        return f

    def ratio_rescan(i, r):
        def f():
            row = np.maximum(t[i] - d[i], np.float32(0.0)) / q[r]
            return int(row.argmax())
        return f

    def t_rescan(i):
        def f():
            return int(t[i].argmax())
        return f

    # ---------------- phase 2 (iterate on greedy frontier) ----------------
    def cdiv(a, b):
        return -(-a // b)

    Kr = len(need_ratio)
    Kt0 = len(frontier)
    RG = max(2, cdiv(cdiv(Kr, NCORES), ROWS_PER_GROUP))
    TG = max(3, cdiv(cdiv(Kt0, NCORES), ROWS_PER_GROUP))
    nc2 = _build_phase2(RG, TG)

    pending_t = [(r, int(starts[r] + frontier[r])) for r in sorted(frontier)]
    ratio_batch = need_ratio
    rounds = 0
    while ratio_batch or pending_t:
        rounds += 1
        if rounds > 2 * S + 2:
            raise RuntimeError("phase-2 did not converge")
        nR, nT = len(ratio_batch), len(pending_t)
        capR, capT = NCORES * RG * ROWS_PER_GROUP, NCORES * TG * ROWS_PER_GROUP
        if nR > capR or nT > capT:
            RG = max(RG, cdiv(cdiv(nR, NCORES), ROWS_PER_GROUP))
            TG = max(TG, cdiv(cdiv(nT, NCORES), ROWS_PER_GROUP))
            nc2 = _build_phase2(RG, TG)

        t2r_h = np.zeros((NCORES, 128, RG * VP), np.float32)
        d2r_h = np.zeros((NCORES, 128, RG * VP), np.float32)
        q2r_h = np.ones((NCORES, 128, RG * VP), np.float32)
        t2t_h = np.zeros((NCORES, 128, TG * VP), np.float32)

        def _stage(dst, m, row_vec):
            c = m % NCORES
            slot = m // NCORES
            g, j = slot // ROWS_PER_GROUP, slot % ROWS_PER_GROUP
            dst[c, j * PARTS_PER_ROW:(j + 1) * PARTS_PER_ROW,
                g * VP:(g + 1) * VP] = row_vec.reshape(PARTS_PER_ROW, VP)

        for m, (r, i) in enumerate(ratio_batch):
            _stage(t2r_h, m, t[i])
            _stage(d2r_h, m, d[i])
            _stage(q2r_h, m, q[r])
        for m, (r, i) in enumerate(pending_t):
            _stage(t2t_h, m, t[i])

        in_maps = [dict(t2r=t2r_h[c], d2r=d2r_h[c], q2r=q2r_h[c], t2t=t2t_h[c])
                   for c in range(NCORES)]
        res2 = _run(nc2, in_maps)

        def _fetch(res_key_m, res_key_i, m):
            c = m % NCORES
            slot = m // NCORES
            g, j = slot // ROWS_PER_GROUP, slot % ROWS_PER_GROUP
            vals = res2[c][res_key_m][j * PARTS_PER_ROW:(j + 1) * PARTS_PER_ROW,
                                      g * 8:(g + 1) * 8]
            idxs = res2[c][res_key_i][j * PARTS_PER_ROW:(j + 1) * PARTS_PER_ROW,
                                      g * 8:(g + 1) * 8].astype(np.int64)
            return vals, idxs

        for m, (r, i) in enumerate(ratio_batch):
            if np.any(q[r] == 0.0):
                # 1/0 on device is undefined enough (0*inf=NaN) — resolve
                # this row entirely on the host (never happens in practice)
                resolved_tok[r] = ratio_rescan(i, r)()
                continue
            vals, idxs = _fetch('m8r', 'i8r', m)
            resolved_tok[r] = _resolve_row(vals, idxs, ratio_exact(i, r),
                                           ratio_rescan(i, r))
        ratio_batch = []

        next_t = []
        for m, (r, i) in enumerate(pending_t):
            vals, idxs = _fetch('m8t', 'i8t', m)
            am = _resolve_t_row(vals, idxs, t_exact(i), t_rescan(i))
            if am == dtid[i]:
                # accepted at this position
                pos = frontier[r] + 1
                if pos < lens[r]:
                    frontier[r] = pos
                    next_t.append((r, int(starts[r] + pos)))
                else:
                    frontier[r] = pos  # fully accepted -> bonus
            else:
                first_rej[r] = frontier[r]
                resolved_tok[r] = am
        pending_t = next_t

    # greedy requests that ran off the end are fully accepted
    for r, pos in frontier.items():
        if first_rej[r] < 0 and pos < lens[r]:
            # should not happen (loop resolves each frontier row)
            raise RuntimeError("unresolved greedy frontier")

    # ---------------- assembly ----------------
    out = np.full((B, S + 1), PLACEHOLDER, np.int32)
    for r in range(B):
        s0, L = starts[r], lens[r]
        fr = first_rej[r]
        if fr < 0:
            out[r, :L] = dtid[s0:s0 + L].astype(np.int32)
            out[r, L] = bonus[r]
        else:
            out[r, :fr] = dtid[s0:s0 + fr].astype(np.int32)
            out[r, fr] = np.int32(resolved_tok[r])
    return out


def _resolve_t_row(vals, idxs, exact_fn, rescan_fn):
    """Exact first-index argmax of a raw-t row: the surrogate values ARE the
    exact values, so candidates are exact ties at the max."""
    vocab = (np.arange(PARTS_PER_ROW)[:, None] * VP + idxs).reshape(-1)
    flat = vals.reshape(-1)
    m = flat.max()
    if (flat == m).sum() > 1:
        # tie at the max anywhere (or >8 ties within one partition):
        # resolve the first-index semantics exactly on the host
        return rescan_fn()
    return int(vocab[flat == m][0])


# revision 16
# speedup vs baseline: 1.2899x; 1.2899x over previous
"""Trainium2 Bass kernel for AscendRejectionSampler (speculative-decoding
rejection sampling), SPMD across 8 NeuronCores.

Strategy (data parallel over requests/tokens, no collectives):

The reference output needs, per request, the accepted prefix of draft tokens
plus ONE "repair" token at the first rejected position:
  - greedy requests emit argmax(target_probs[row]) there,
  - non-greedy requests emit argmax(relu(t-d)/q) ("recovered") there.
Accept bits need only single-element gathers t[i, dtid], d[i, dtid] (random)
or the row argmax itself (greedy).  So the expensive full-vocab scans are
needed for ~1 row per request, not for every token.

Phase 1 (NEFF 1): per-token 64-wide chunks containing dtid are staged by the
host; the device extracts t_at/d_at via masked tensor_tensor_reduce and
computes the random-accept bits exactly ((d>0) & (t >= u*d), IEEE f32).

Host: per-request scan over accept bits (control flow) picks the first
rejected position; greedy requests start with frontier position 0.

Phase 2 (NEFF 2): the needed rows are staged split 16-partitions-per-row
(8 rows per 128-partition group, 2000 vocab/partition).  Ratio rows run
(t - d) / q with exact IEEE divide on the vector engine, then per-partition
top-8 (max) + indices (max_index).  Greedy rows take top-8 of raw t.  The
host merges the 128 candidates per row and resolves the exact first-index
argmax (ties included) by re-checking only the few candidate elements.
If a greedy frontier row is *accepted* (P ~ 1/32000 per row), the frontier
advances and NEFF 2 reruns for those few rows.

All O(N*V) data processing happens on-device; the host does control flow,
staging/unstaging, and O(candidates) exact tie resolution.
"""

import sys

if '/opt/trn_rl_repo' not in sys.path:
    sys.path.insert(0, '/opt/trn_rl_repo')

import numpy as np

NCORES = 8
PLACEHOLDER = -1
VP = 2000          # vocab elements per partition in phase 2 (16 partitions/row)
PARTS_PER_ROW = 16
ROWS_PER_GROUP = 128 // PARTS_PER_ROW  # 8

# Collected exec_time_ns per NEFF execution when profiling is enabled.
PROFILE = False
LAST_EXEC_NS = []

_BUILT = {}


def _bass_mods():
    import concourse.mybir as mybir
    from concourse import bass
    from concourse.bass_utils import run_bass_kernel_spmd
    return mybir, bass, run_bass_kernel_spmd


def _maybe_install_ntff_hook():
    """Register the axon NTFF profile hook if the image lacks antenv.axon_hooks."""
    import types
    try:
        import antenv.axon_hooks  # noqa: F401
        return
    except ImportError:
        pass
    import antenv
    mod = types.ModuleType('antenv.axon_hooks')
    _h = [None]
    mod.set_axon_ntff_profile_hook = lambda h: _h.__setitem__(0, h)
    mod.get_axon_ntff_profile_hook = lambda: _h[0]
    sys.modules['antenv.axon_hooks'] = mod
    antenv.axon_hooks = mod
    try:
        from trn_agent_boot.trn_boot import _ntff_profile_via_ctypes
        mod.set_axon_ntff_profile_hook(
            _ntff_profile_via_ctypes('/opt/axon/libaxon_pjrt.so'))
    except Exception:
        pass


def _run(nc, in_maps):
    _, _, run_bass_kernel_spmd = _bass_mods()
    if PROFILE:
        _maybe_install_ntff_hook()
        res = run_bass_kernel_spmd(nc, in_maps, core_ids=list(range(NCORES)),
                                   trace=True)
        if res.exec_time_ns is not None:
            LAST_EXEC_NS.append(res.exec_time_ns)
        return res.results
    res = run_bass_kernel_spmd(nc, in_maps, core_ids=list(range(NCORES)))
    return res.results


# --------------------------------------------------------------------------
# Phase 1: single-element gathers + random-accept bits
# --------------------------------------------------------------------------

def _build_phase1(SL1):
    """Per core: [128, SL1] token slots. One packed input [128, 3F + SL1]:
    [ct | cd | em | uu] where ct/cd are 64-elem chunks of t/d containing
    dtid, em is the one-hot extract mask, uu the uniforms.  One packed
    output [128, 3*SL1]: [ta | da | bit]."""
    key = ('p1', SL1)
    if key in _BUILT:
        return _BUILT[key]
    mybir, bass, _ = _bass_mods()
    DT = mybir.dt.float32
    F = SL1 * 64
    FP = 3 * F + SL1
    nc = bass.Bass()
    pk = nc.declare_dram_parameter("pk", [128, FP], DT, isOutput=False)
    out_o = nc.declare_dram_parameter("out", [128, 3 * SL1], DT, isOutput=True)

    with (
        nc.Block() as block,
        nc.semaphore("dma_sem") as dma_sem,
        nc.semaphore("v_sem") as v_sem,
        nc.sbuf_tensor("pk_sb", [128, FP], DT) as pk_sb,
        nc.sbuf_tensor("junk_sb", [128, 128 * SL1], DT) as junk_sb,
        nc.sbuf_tensor("o_sb", [128, 3 * SL1], DT) as o_sb,
        nc.sbuf_tensor("t2_sb", [128, SL1], DT) as t2_sb,
        nc.sbuf_tensor("t3_sb", [128, SL1], DT) as t3_sb,
        nc.sbuf_tensor("ge_sb", [128, SL1], DT) as ge_sb,
    ):
        ct_sb = pk_sb[:, 0:F]
        cd_sb = pk_sb[:, F:2 * F]
        em_sb = pk_sb[:, 2 * F:3 * F]
        uu_sb = pk_sb[:, 3 * F:3 * F + SL1]
        ta_sb = o_sb[:, 0:SL1]
        da_sb = o_sb[:, SL1:2 * SL1]
        bit_sb = o_sb[:, 2 * SL1:3 * SL1]

        @block.sync
        def _(sync):
            sync.dma_start(out=pk_sb[:, :], in_=pk[:, :]).then_inc(dma_sem, 16)
            sync.wait_ge(v_sem, 1)
            sync.dma_start(out=out_o[:, :], in_=o_sb[:, :]).then_inc(dma_sem, 16)
            sync.wait_ge(dma_sem, 32)

        @block.vector
        def _(v):
            v.wait_ge(dma_sem, 16)
            A = mybir.AluOpType
            for sl in range(SL1):
                c = slice(sl * 64, (sl + 1) * 64)
                v.scalar_tensor_tensor(
                    junk_sb[:, 128 * sl:128 * sl + 64], ct_sb[:, c], 1.0,
                    em_sb[:, c], A.mult, A.mult,
                    accum_out=ta_sb[:, sl:sl + 1])
                v.scalar_tensor_tensor(
                    junk_sb[:, 128 * sl + 64:128 * (sl + 1)], cd_sb[:, c], 1.0,
                    em_sb[:, c], A.mult, A.mult,
                    accum_out=da_sb[:, sl:sl + 1])
            v.drain()
            # t2 = u * da ; t3 = (da > 0)
            v.tensor_tensor(t2_sb[:, :], uu_sb[:, :], da_sb[:, :], A.mult)
            v.tensor_scalar(t3_sb[:, :], da_sb[:, :], 0.0, None, A.is_gt)
            v.drain()
            # ge = (ta >= u*da)
            v.tensor_tensor(ge_sb[:, :], ta_sb[:, :], t2_sb[:, :], A.is_ge)
            v.drain()
            v.tensor_tensor(bit_sb[:, :], t3_sb[:, :], ge_sb[:, :],
                            A.logical_and)
            v.drain()
            v.sem_inc(v_sem, 1)

    _BUILT[key] = nc
    return nc


# --------------------------------------------------------------------------
# Phase 2: row argmax candidates (top-8 per 16-partition slice of each row)
# --------------------------------------------------------------------------

def _build_phase2(RG, TG):
    """Per core: RG groups of 8 ratio rows ((t-d)/q) and TG groups of 8
    plain-t rows. Each row is split over 16 partitions x VP elements.
    Outputs per-partition top-8 values + indices for every group."""
    key = ('p2', RG, TG)
    if key in _BUILT:
        return _BUILT[key]
    mybir, bass, _ = _bass_mods()
    DT = mybir.dt.float32
    DU = mybir.dt.uint32
    FR = RG * VP
    FT = TG * VP
    nc = bass.Bass()
    t2r = nc.declare_dram_parameter("t2r", [128, FR], DT, isOutput=False)
    d2r = nc.declare_dram_parameter("d2r", [128, FR], DT, isOutput=False)
    q2r = nc.declare_dram_parameter("q2r", [128, FR], DT, isOutput=False)
    t2t = nc.declare_dram_parameter("t2t", [128, FT], DT, isOutput=False)
    m8r_o = nc.declare_dram_parameter("m8r", [128, RG * 8], DT, isOutput=True)
    i8r_o = nc.declare_dram_parameter("i8r", [128, RG * 8], DU, isOutput=True)
    m8t_o = nc.declare_dram_parameter("m8t", [128, TG * 8], DT, isOutput=True)
    i8t_o = nc.declare_dram_parameter("i8t", [128, TG * 8], DU, isOutput=True)

    with (
        nc.Block() as block,
        nc.semaphore("dma_sem") as dma_sem,
        nc.semaphore("v_sem") as v_sem,
        nc.semaphore("s_sem") as s_sem,
        nc.sbuf_tensor("t2t_sb", [128, FT], DT) as t2t_sb,
        nc.sbuf_tensor("t2r_sb", [128, FR], DT) as t2r_sb,
        nc.sbuf_tensor("d2r_sb", [128, FR], DT) as d2r_sb,
        nc.sbuf_tensor("q2r_sb", [128, FR], DT) as q2r_sb,
        nc.sbuf_tensor("rcp_sb", [128, FR], DT) as rcp_sb,
        nc.sbuf_tensor("s_sb", [128, FR], DT) as s_sb,
        nc.sbuf_tensor("m8r_sb", [128, RG * 8], DT) as m8r_sb,
        nc.sbuf_tensor("i8r_sb", [128, RG * 8], DU) as i8r_sb,
        nc.sbuf_tensor("m8t_sb", [128, TG * 8], DT) as m8t_sb,
        nc.sbuf_tensor("i8t_sb", [128, TG * 8], DU) as i8t_sb,
    ):
        @block.sync
        def _(sync):
            sync.dma_start(out=t2t_sb[:, :], in_=t2t[:, :]).then_inc(dma_sem, 16)
            sync.dma_start(out=q2r_sb[:, :], in_=q2r[:, :]).then_inc(dma_sem, 16)
            sync.dma_start(out=t2r_sb[:, :], in_=t2r[:, :]).then_inc(dma_sem, 16)
            sync.dma_start(out=d2r_sb[:, :], in_=d2r[:, :]).then_inc(dma_sem, 16)
            sync.wait_ge(v_sem, 1)
            sync.dma_start(out=m8t_o[:, :], in_=m8t_sb[:, :]).then_inc(dma_sem, 16)
            sync.dma_start(out=i8t_o[:, :], in_=i8t_sb[:, :]).then_inc(dma_sem, 16)
            sync.wait_ge(v_sem, 2)
            sync.dma_start(out=m8r_o[:, :], in_=m8r_sb[:, :]).then_inc(dma_sem, 16)
            sync.dma_start(out=i8r_o[:, :], in_=i8r_sb[:, :]).then_inc(dma_sem, 16)
            sync.wait_ge(dma_sem, 128)

        @block.scalar
        def _(s):
            # approximate 1/q on the Scalar engine (~1.2e-5 rel in range;
            # the host candidate window + q-range guard absorb the error)
            s.wait_ge(dma_sem, 32)
            inputs = [s.lower_ap(q2r_sb[:, :])]
            for val in (0.0, 1.0, 0.0):  # bias, scale, alpha
                inputs.append(mybir.ImmediateValue(dtype=mybir.dt.float32,
                                                   value=val))
            s.add_instruction(mybir.InstActivation(
                name=s.bass.get_next_instruction_name(),
                func=mybir.ActivationFunctionType.Reciprocal,
                ins=inputs,
                outs=[s.lower_ap(rcp_sb[:, :])],
            ))
            s.drain()
            s.sem_inc(s_sem, 1)

        @block.vector
        def _(v):
            A = mybir.AluOpType
            # t pipe first: only needs t2t (first DMA)
            v.wait_ge(dma_sem, 16)
            for g in range(TG):
                v.max(m8t_sb[:, g * 8:(g + 1) * 8],
                      t2t_sb[:, g * VP:(g + 1) * VP])
            v.drain()
            for g in range(TG):
                v.max_index(i8t_sb[:, g * 8:(g + 1) * 8],
                            m8t_sb[:, g * 8:(g + 1) * 8],
                            t2t_sb[:, g * VP:(g + 1) * VP])
            v.drain()
            v.sem_inc(v_sem, 1)
            # ratio pipe: s = (t - d) * scalar_recip(q)
            v.wait_ge(dma_sem, 64)
            v.tensor_tensor(s_sb[:, :], t2r_sb[:, :], d2r_sb[:, :], A.subtract)
            v.drain()
            v.wait_ge(s_sem, 1)
            v.tensor_tensor(d2r_sb[:, :], s_sb[:, :], rcp_sb[:, :], A.mult)
            v.drain()
            for g in range(RG):
                v.max(m8r_sb[:, g * 8:(g + 1) * 8],
                      d2r_sb[:, g * VP:(g + 1) * VP])
            v.drain()
            for g in range(RG):
                v.max_index(i8r_sb[:, g * 8:(g + 1) * 8],
                            m8r_sb[:, g * 8:(g + 1) * 8],
                            d2r_sb[:, g * VP:(g + 1) * VP])
            v.drain()
            v.sem_inc(v_sem, 1)

    _BUILT[key] = nc
    return nc


# --------------------------------------------------------------------------
# Host-side exact candidate resolution
# --------------------------------------------------------------------------

def _resolve_row(vals, idxs, exact_fn, rescan_fn, reltol=1e-3):
    """vals/idxs: [16, 8] per-partition top-8 (desc) of one row's surrogate,
    idxs are positions within each partition's VP slice. Returns the exact
    first-index argmax of the true row.

    exact_fn(vocab_idx_array) -> exact f32 values (reference arithmetic).
    rescan_fn() -> exact full-row argmax fallback (rare)."""
    vocab = (np.arange(PARTS_PER_ROW)[:, None] * VP + idxs).reshape(-1)
    flat = vals.reshape(-1)
    ms = float(flat.max())
    thr = ms - abs(ms) * reltol
    # truncation guard: a partition whose 8th-best still clears the threshold
    # may have deeper candidates we did not see
    if np.any(vals[:, 7] >= thr):
        return rescan_fn()
    # duplicate guard: equal surrogate values inside one partition's top-8
    # near the max — max_index tie semantics may hide one of the positions
    if np.any((vals[:, :-1] == vals[:, 1:]) & (vals[:, 1:] >= thr)):
        return rescan_fn()
    sel = flat >= thr
    cand_v = vocab[sel]
    exact = exact_fn(cand_v)
    me = exact.max()
    if not (me > 0.0):
        # all-candidates <= 0: the true argmax may be an arbitrary early
        # zero/subnormal position the surrogate can't distinguish
        return rescan_fn()
    winners = cand_v[exact == me]
    if len(winners) > 1:
        # exact ties across candidates: be conservative about capture
        return rescan_fn()
    return int(winners[0])


# --------------------------------------------------------------------------
# The kernel
# --------------------------------------------------------------------------

def kernel(**inputs):
    t = np.ascontiguousarray(np.asarray(inputs['target_probs'], dtype=np.float32))
    d = np.ascontiguousarray(np.asarray(inputs['draft_probs'], dtype=np.float32))
    q = np.ascontiguousarray(np.asarray(inputs['q'], dtype=np.float32))
    u = np.asarray(inputs['uniform_probs'], dtype=np.float32)
    cu = np.asarray(inputs['cu_num_draft_tokens']).astype(np.int64)
    dtid = np.asarray(inputs['draft_token_ids']).astype(np.int64)
    bonus = np.asarray(inputs['bonus_token_ids']).astype(np.int32)
    greedy = np.asarray(inputs['is_greedy']).astype(bool)
    S = int(np.asarray(inputs['max_spec_len']))

    N, V = t.shape
    B = cu.shape[0]
    assert V % (PARTS_PER_ROW * VP) == 0 or V == PARTS_PER_ROW * VP, \
        f"V={V} not supported"
    starts = np.concatenate([[0], cu[:-1]]).astype(np.int64)
    lens = (cu - starts).astype(np.int64)
    tok_req = np.searchsorted(cu, np.arange(N), side='right')

    # ---------------- phase 1 ----------------
    T1 = -(-N // NCORES)                   # tokens per core
    SL1 = max(2, -(-T1 // 128))            # 128-token slots per core
    nc1 = _build_phase1(SL1)

    F = SL1 * 64
    pk_h = np.zeros((NCORES, 128, 3 * F + SL1), np.float32)

    ii = np.arange(N)
    core_of = ii // T1
    loc = ii - core_of * T1
    p_of = loc % 128
    sl_of = loc // 128
    base = (dtid // 64) * 64
    off = (dtid % 64).astype(np.int64)
    # chunk gathers (contiguous 64-element slices -> cheap row slicing)
    cols = base[:, None] + np.arange(64)[None, :]
    tch = np.take_along_axis(t, cols, axis=1)
    dch = np.take_along_axis(d, cols, axis=1)
    # vectorized scatter into the packed staging array [ct | cd | em | uu]
    col0 = sl_of * 64
    for k in range(64):
        pk_h[core_of, p_of, col0 + k] = tch[:, k]
        pk_h[core_of, p_of, F + col0 + k] = dch[:, k]
    pk_h[core_of, p_of, 2 * F + col0 + off] = 1.0
    pk_h[core_of, p_of, 3 * F + sl_of] = u

    in_maps = [dict(pk=pk_h[c]) for c in range(NCORES)]
    res1 = _run(nc1, in_maps)

    ta = np.zeros(N, np.float32)
    da = np.zeros(N, np.float32)
    bits = np.zeros(N, bool)
    for c in range(NCORES):
        lo, hi = c * T1, min(N, (c + 1) * T1)
        n = hi - lo
        o = res1[c]['out']
        ta[lo:hi] = o[:, 0:SL1].T.reshape(-1)[:n]
        da[lo:hi] = o[:, SL1:2 * SL1].T.reshape(-1)[:n]
        bits[lo:hi] = o[:, 2 * SL1:3 * SL1].T.reshape(-1)[:n] != 0.0

    # ---------------- host scan (control flow) ----------------
    # random requests: first rejected position from accept bits
    first_rej = np.full(B, -1, np.int64)   # -1 = no rejection so far
    resolved_tok = np.full(B, PLACEHOLDER, np.int64)
    need_ratio = []                        # (req, token_row)
    frontier = {}                          # greedy req -> current position
    for r in range(B):
        s0, L = starts[r], lens[r]
        if greedy[r]:
            frontier[r] = 0
        else:
            rej = np.nonzero(~bits[s0:s0 + L])[0]
            if len(rej):
                first_rej[r] = rej[0]
                need_ratio.append((r, int(s0 + rej[0])))

    # exact reference arithmetic helpers (single-element touches only)
    def ratio_exact(i, r):
        def f(vs):
            num = np.maximum(t[i, vs] - d[i, vs], np.float32(0.0))
            return (num / q[r, vs]).astype(np.float32)
        return f

    def t_exact(i):
        def f(vs):
            return t[i, vs]
        return f

    def ratio_rescan(i, r):
        def f():
            row = np.maximum(t[i] - d[i], np.float32(0.0)) / q[r]
            return int(row.argmax())
        return f

    def t_rescan(i):
        def f():
            return int(t[i].argmax())
        return f

    # ---------------- phase 2 (iterate on greedy frontier) ----------------
    def cdiv(a, b):
        return -(-a // b)

    Kr = len(need_ratio)
    Kt0 = len(frontier)
    RG = max(2, cdiv(cdiv(Kr, NCORES), ROWS_PER_GROUP))
    TG = max(3, cdiv(cdiv(Kt0, NCORES), ROWS_PER_GROUP))
    nc2 = _build_phase2(RG, TG)

    pending_t = [(r, int(starts[r] + frontier[r])) for r in sorted(frontier)]
    ratio_batch = need_ratio
    rounds = 0
    while ratio_batch or pending_t:
        rounds += 1
        if rounds > 2 * S + 2:
            raise RuntimeError("phase-2 did not converge")
        nR, nT = len(ratio_batch), len(pending_t)
        capR, capT = NCORES * RG * ROWS_PER_GROUP, NCORES * TG * ROWS_PER_GROUP
        if nR > capR or nT > capT:
            RG = max(RG, cdiv(cdiv(nR, NCORES), ROWS_PER_GROUP))
            TG = max(TG, cdiv(cdiv(nT, NCORES), ROWS_PER_GROUP))
            nc2 = _build_phase2(RG, TG)

        t2r_h = np.zeros((NCORES, 128, RG * VP), np.float32)
        d2r_h = np.zeros((NCORES, 128, RG * VP), np.float32)
        q2r_h = np.ones((NCORES, 128, RG * VP), np.float32)
        t2t_h = np.zeros((NCORES, 128, TG * VP), np.float32)

        def _stage(dst, m, row_vec):
            c = m % NCORES
            slot = m // NCORES
            g, j = slot // ROWS_PER_GROUP, slot % ROWS_PER_GROUP
            dst[c, j * PARTS_PER_ROW:(j + 1) * PARTS_PER_ROW,
                g * VP:(g + 1) * VP] = row_vec.reshape(PARTS_PER_ROW, VP)

        for m, (r, i) in enumerate(ratio_batch):
            _stage(t2r_h, m, t[i])
            _stage(d2r_h, m, d[i])
            _stage(q2r_h, m, q[r])
        for m, (r, i) in enumerate(pending_t):
            _stage(t2t_h, m, t[i])

        in_maps = [dict(t2r=t2r_h[c], d2r=d2r_h[c], q2r=q2r_h[c], t2t=t2t_h[c])
                   for c in range(NCORES)]
        res2 = _run(nc2, in_maps)

        def _fetch(res_key_m, res_key_i, m):
            c = m % NCORES
            slot = m // NCORES
            g, j = slot // ROWS_PER_GROUP, slot % ROWS_PER_GROUP
            vals = res2[c][res_key_m][j * PARTS_PER_ROW:(j + 1) * PARTS_PER_ROW,
                                      g * 8:(g + 1) * 8]
            idxs = res2[c][res_key_i][j * PARTS_PER_ROW:(j + 1) * PARTS_PER_ROW,
                                      g * 8:(g + 1) * 8].astype(np.int64)
            return vals, idxs

        for m, (r, i) in enumerate(ratio_batch):
            if np.min(np.abs(q[r])) < 1e-6 or np.max(np.abs(q[r])) > 1e9:
                # the Scalar-engine reciprocal LUT misbehaves far outside
                # [1e-10, 1e10] (and 1/0 is undefined) — resolve such rows
                # on the host (a handful per call at most)
                resolved_tok[r] = ratio_rescan(i, r)()
                continue
            vals, idxs = _fetch('m8r', 'i8r', m)
            resolved_tok[r] = _resolve_row(vals, idxs, ratio_exact(i, r),
                                           ratio_rescan(i, r))
        ratio_batch = []

        next_t = []
        for m, (r, i) in enumerate(pending_t):
            vals, idxs = _fetch('m8t', 'i8t', m)
            am = _resolve_t_row(vals, idxs, t_exact(i), t_rescan(i))
            if am == dtid[i]:
                # accepted at this position
                pos = frontier[r] + 1
                if pos < lens[r]:
                    frontier[r] = pos
                    next_t.append((r, int(starts[r] + pos)))
                else:
                    frontier[r] = pos  # fully accepted -> bonus
            else:
                first_rej[r] = frontier[r]
                resolved_tok[r] = am
        pending_t = next_t

    # greedy requests that ran off the end are fully accepted
    for r, pos in frontier.items():
        if first_rej[r] < 0 and pos < lens[r]:
            # should not happen (loop resolves each frontier row)
            raise RuntimeError("unresolved greedy frontier")

    # ---------------- assembly ----------------
    out = np.full((B, S + 1), PLACEHOLDER, np.int32)
    for r in range(B):
        s0, L = starts[r], lens[r]
        fr = first_rej[r]
        if fr < 0:
            out[r, :L] = dtid[s0:s0 + L].astype(np.int32)
            out[r, L] = bonus[r]
        else:
            out[r, :fr] = dtid[s0:s0 + fr].astype(np.int32)
            out[r, fr] = np.int32(resolved_tok[r])
    return out


def _resolve_t_row(vals, idxs, exact_fn, rescan_fn):
    """Exact first-index argmax of a raw-t row: the surrogate values ARE the
    exact values, so candidates are exact ties at the max."""
    vocab = (np.arange(PARTS_PER_ROW)[:, None] * VP + idxs).reshape(-1)
    flat = vals.reshape(-1)
    m = flat.max()
    if (flat == m).sum() > 1:
        # tie at the max anywhere (or >8 ties within one partition):
        # resolve the first-index semantics exactly on the host
        return rescan_fn()
    return int(vocab[flat == m][0])
